# revision 29
# baseline (speedup 1.0000x reference)
"""AGNN conv kernel for trn2: out = x + relu(agnn_conv(x, edge_index, beta)).

Strategy: destination-sharded edge partitioning across 8 NeuronCores.
Host-side index preprocessing builds a padded CSR (incoming src lists per
node, incl. self loop) with nodes packed into degree-homogeneous tiles of
128 (one node per SBUF partition). Each core gathers [x | x_norm] rows of
its slot table from HBM via dma_gather (256B rows), computes the cosine
attention softmax densely per partition, and writes its output rows.
No cross-core communication is needed (all edges of a node live on one
core); the host re-assembles the full output.

The x_norm half of the gather table is computed on device in a prologue.
int16 gather indices only span 32k rows, so the table is split into lo/hi
halves (row < HALF vs >= HALF) and every node tile carries two slot
structures, one per half. Tiles are half-pure so the node's own (self)
row sits at slot 0 of its own half's structure.
"""

import sys
import numpy as np

sys.path.insert(0, '/opt/trn_rl_repo')

N_CORES = 8
P = 128           # SBUF partitions / nodes per tile
T = 2             # node tiles per supertile (one gather pair per supertile)
D = 32            # feature dim
ELEM = 64         # f32 per table row: [x(32) | xh(32)] = 256B
K_ROUND = 1       # round slot counts up to a multiple of this
PAD_BIAS = -1.0e9
OFFLOAD_POOL = {'dm0', 'dm1', 'p0', 'p1'}  # muls to run on GPSIMD
BALANCE_DM0_DVE = 1  # of every 5 rounds, run dm0 on DVE this many
BALANCE_P0_DVE = 0   # same for the lo P-mul
PRO_BUFS = 6
PRO_CHUNKS = 14
DM_BF16 = False        # write product tiles in bf16 (faster single-src reduce)


# ---------------------------------------------------------------------------
# host-side index preprocessing
# ---------------------------------------------------------------------------

def preprocess(edge_index, n_nodes):
    """Build per-core gather/bias planes and node lists.

    Returns a dict with everything the device program and the output
    unshuffle need. Only integer index manipulation happens here.
    """
    n_pad = -(-n_nodes // P) * P            # table rows padded to 128
    half = n_pad // 2                        # lo rows [0, half), hi [half, 2*half)
    src = np.asarray(edge_index[0], dtype=np.int64)
    dst = np.asarray(edge_index[1], dtype=np.int64)

    is_hi = src >= half
    # CSR of incoming edges per (dst, half): sort by dst with lo srcs first
    key = dst * 2 + is_hi
    order = np.argsort(key, kind='stable')
    src_sorted = src[order]
    d_lo = np.bincount(dst[~is_hi], minlength=n_nodes)
    d_hi = np.bincount(dst[is_hi], minlength=n_nodes)
    deg = d_lo + d_hi
    # starts[i] = first edge of node i in src_sorted; lo edges then hi edges
    starts = np.zeros(n_nodes + 1, dtype=np.int64)
    np.cumsum(deg, out=starts[1:])

    node_is_hi = np.arange(n_nodes) >= half
    slots_lo = d_lo + (~node_is_hi)          # self slot for lo nodes
    slots_hi = d_hi + node_is_hi

    sup_nodes = P * T

    def build_half(ids_real):
        """Order one half's nodes into padded supertiles; return node array.

        Sort key buckets d_lo coarsely then orders by d_hi so that both
        per-tile maxima stay close to the per-node values (minimises padded
        slots; a plain total-degree sort leaves the lo/hi binomial split
        unpacked and costs ~15% more gather traffic).
        """
        a = slots_lo[ids_real].astype(np.int64)
        b = slots_hi[ids_real].astype(np.int64)
        key = np.maximum(a, b) * 100000 + np.minimum(a, b)
        ids_sorted = ids_real[np.argsort(key, kind='stable')]
        n_sup = -(-len(ids_sorted) // sup_nodes)
        n_sup = -(-n_sup // N_CORES) * N_CORES          # rounds of 8
        padded = np.full(n_sup * sup_nodes, ids_sorted[0], dtype=np.int64)
        padded[:len(ids_sorted)] = ids_sorted
        real = np.zeros(n_sup * sup_nodes, dtype=bool)
        real[:len(ids_sorted)] = True
        return padded.reshape(n_sup, sup_nodes), real.reshape(n_sup, sup_nodes)

    lo_ids = np.arange(0, min(half, n_nodes))
    hi_ids = np.arange(half, n_nodes)
    sup_lo, real_lo = build_half(lo_ids)
    sup_hi, real_hi = build_half(hi_ids)

    # deal supertiles (sorted by cost desc) round-robin to cores;
    # every core's round j shares the max K of the 8 supertiles in it.
    def deal(sup, real):
        cost = np.array([
            max(slots_lo[s].max(), slots_hi[s].max()) for s in sup])
        o = np.argsort(-cost, kind='stable')
        sup, real = sup[o], real[o]
        n_rounds = len(sup) // N_CORES
        rounds = []
        for j in range(n_rounds):
            grp = sup[j * N_CORES:(j + 1) * N_CORES]
            grp_real = real[j * N_CORES:(j + 1) * N_CORES]
            kl = int(max(slots_lo[g].max() for g in grp))
            kh = int(max(slots_hi[g].max() for g in grp))
            kl = max(1, -(-kl // K_ROUND) * K_ROUND)
            kh = max(1, -(-kh // K_ROUND) * K_ROUND)
            rounds.append((grp, grp_real, kl, kh))
        return rounds

    rounds = deal(sup_lo, real_lo) + deal(sup_hi, real_hi)
    n_lo_rounds = sup_lo.shape[0] // N_CORES

    col = None  # lazily sized scratch

    def slot_matrix(ids, own_half_is_hi, want_hi):
        """[len(ids), K] int16 slot matrix + bias for one structure."""
        nonlocal col
        n = len(ids)
        base = half if want_hi else 0
        if want_hi:
            cnt_edges = d_hi[ids]
            edge_start = starts[ids] + d_lo[ids]
        else:
            cnt_edges = d_lo[ids]
            edge_start = starts[ids]
        own = own_half_is_hi == want_hi
        self_off = 1 if own else 0
        cnt = cnt_edges + self_off
        K = int(cnt.max())
        K = max(1, -(-K // K_ROUND) * K_ROUND)
        S = np.zeros((n, K), dtype=np.int64)
        cols = np.arange(K)[None, :]
        valid = cols < cnt[:, None]
        if own:
            S[:, 0] = ids - base
            e_col = cols - 1
        else:
            e_col = cols
        take = edge_start[:, None] + e_col
        e_valid = valid & (e_col >= 0)
        S[e_valid] = src_sorted[np.clip(take, 0, len(src_sorted) - 1)][e_valid] - base
        bias = np.where(valid, 0.0, PAD_BIAS).astype(np.float32)
        return S.astype(np.int16), bias, K

    # per-core streams
    cores = [{'idx_lo': [], 'idx_hi': [], 'b_lo': [], 'b_hi': [],
              'nodes': [], 'real': []} for _ in range(N_CORES)]
    shapes = []  # (kl, kh) per round, shared across cores

    def wrap16(L):
        # unwrapped[j] = plane[j % 16, j // 16]; replicate over 8 groups
        plane = L.reshape(-1, 16).T.copy()
        return np.tile(plane, (8, 1))

    for j, (grp, grp_real, kl, kh) in enumerate(rounds):
        own_hi = j >= n_lo_rounds
        shapes.append((kl, kh))
        for c in range(N_CORES):
            ids = grp[c]
            S_lo, B_lo, _ = pad_to(slot_matrix(ids, own_hi, False), kl)
            S_hi, B_hi, _ = pad_to(slot_matrix(ids, own_hi, True), kh)
            # index order j = g*128 + p with g = t*K + k
            L_lo = S_lo.reshape(T, P, kl).transpose(0, 2, 1).reshape(-1)
            L_hi = S_hi.reshape(T, P, kh).transpose(0, 2, 1).reshape(-1)
            cores[c]['idx_lo'].append(wrap16(L_lo))
            cores[c]['idx_hi'].append(wrap16(L_hi))
            # bias planes [128, T*K] in [p, t*K + k] layout
            cores[c]['b_lo'].append(
                B_lo.reshape(T, P, kl).transpose(1, 0, 2).reshape(P, T * kl))
            cores[c]['b_hi'].append(
                B_hi.reshape(T, P, kh).transpose(1, 0, 2).reshape(P, T * kh))
            cores[c]['nodes'].append(grp[c])
            cores[c]['real'].append(grp_real[c])

    for c in range(N_CORES):
        cc = cores[c]
        cc['idx_lo'] = np.concatenate(cc['idx_lo'], axis=1)
        cc['idx_hi'] = np.concatenate(cc['idx_hi'], axis=1)
        cc['b_lo'] = np.concatenate(cc['b_lo'], axis=1)
        cc['b_hi'] = np.concatenate(cc['b_hi'], axis=1)
        cc['nodes'] = np.concatenate(cc['nodes'])
        cc['real'] = np.concatenate(cc['real'])

    return {
        'cores': cores, 'shapes': shapes, 'n_lo_rounds': n_lo_rounds,
        'half': half, 'n_pad': n_pad,
    }


def pad_to(smb, K):
    """Pad a (S, bias, k) triple's columns out to K."""
    S, B, k = smb
    if k == K:
        return S, B, K
    assert k < K
    n = S.shape[0]
    S2 = np.zeros((n, K), dtype=np.int16)
    S2[:, :k] = S
    B2 = np.full((n, K), PAD_BIAS, dtype=np.float32)
    B2[:, :k] = B
    return S2, B2, K


# ---------------------------------------------------------------------------
# numpy emulation of the device program (for validation)
# ---------------------------------------------------------------------------

def emulate(x, beta, pre):
    n_nodes = x.shape[0]
    half, n_pad = pre['half'], pre['n_pad']
    xt = np.zeros((n_pad, ELEM), dtype=np.float32)
    xt[:n_nodes, :D] = x
    # device prologue: xh = x * 1/sqrt(sum(x^2) + 1e-30)
    s = (xt[:, :D] ** 2).sum(-1) + 1e-30
    inv_n = np.sqrt((1.0 / s)).astype(np.float32)
    xt[:, D:] = xt[:, :D] * inv_n[:, None]

    b = float(beta[0])
    out_full = np.zeros((n_nodes, D), dtype=np.float32)
    shapes = pre['shapes']
    n_lo_rounds = pre['n_lo_rounds']

    for c in range(N_CORES):
        cc = pre['cores'][c]
        off_il = off_ih = off_bl = off_bh = 0
        outs = []
        for j, (kl, kh) in enumerate(shapes):
            own_hi = j >= n_lo_rounds
            nil, nih = P * T * kl, P * T * kh
            plane_l = cc['idx_lo'][:16, off_il:off_il + nil // 16]
            plane_h = cc['idx_hi'][:16, off_ih:off_ih + nih // 16]
            off_il += nil // 16; off_ih += nih // 16
            L_lo = plane_l.T.reshape(-1)
            L_hi = plane_h.T.reshape(-1)
            B_lo = cc['b_lo'][:, off_bl:off_bl + T * kl]; off_bl += T * kl
            B_hi = cc['b_hi'][:, off_bh:off_bh + T * kh]; off_bh += T * kh
            # gather: G[p, g, :] = slice[L[g*128+p]]
            G_lo = xt[:half][L_lo.reshape(T * kl, P).T.astype(np.int64)]
            G_hi = xt[half:][L_hi.reshape(T * kh, P).T.astype(np.int64)]
            G_lo = G_lo.reshape(P, T, kl, ELEM)
            G_hi = G_hi.reshape(P, T, kh, ELEM)
            G_own = G_hi if own_hi else G_lo
            xh_self = G_own[:, :, 0, D:]                      # [P, T, 32]
            x_self = G_own[:, :, 0, :D]
            dot_lo = (G_lo[:, :, :, D:] * xh_self[:, :, None, :]).sum(-1)
            dot_hi = (G_hi[:, :, :, D:] * xh_self[:, :, None, :]).sum(-1)
            a_lo = dot_lo + B_lo.reshape(P, T, kl)
            a_hi = dot_hi + B_hi.reshape(P, T, kh)
            e_lo = np.exp(b * a_lo - b)
            e_hi = np.exp(b * a_hi - b)
            den = e_lo.sum(-1) + e_hi.sum(-1)                 # [P, T]
            agg = (e_lo[..., None] * G_lo[:, :, :, :D]).sum(2) \
                + (e_hi[..., None] * G_hi[:, :, :, :D]).sum(2)
            o = x_self + np.maximum(agg / den[..., None], 0.0)
            outs.append(o.transpose(1, 0, 2).reshape(T * P, D))
        out_c = np.concatenate(outs, 0)
        m = cc['real']
        out_full[cc['nodes'][m]] = out_c[m]
    return out_full


# ---------------------------------------------------------------------------
# device program
# ---------------------------------------------------------------------------

_PROG_CACHE = {}


def build_program(shapes, n_lo_rounds, half, n_pad, w_stream):
    import concourse.bass as bass
    import concourse.bacc as bacc
    import concourse.tile as tile
    from concourse import mybir

    f32 = mybir.dt.float32
    i16 = mybir.dt.int16
    Alu = mybir.AluOpType
    Act = mybir.ActivationFunctionType
    n_rounds = len(shapes)
    n_out = n_rounds * T * P

    nc = bacc.Bacc()
    xt = nc.declare_dram_parameter('xt', [n_pad, ELEM], f32, isOutput=False)
    xc = nc.declare_dram_parameter('xc', [n_pad, D], f32, isOutput=False)
    stream = nc.declare_dram_parameter('stream', [P, w_stream], i16, isOutput=False)
    beta_in = nc.declare_dram_parameter('beta', [P, 1], f32, isOutput=False)
    out = nc.declare_dram_parameter('out', [n_out, D], f32, isOutput=True)

    NT = n_pad // P                            # t-cols per partition (e.g. 391)
    xt_c = xt[:].rearrange('(p t) e -> p (t e)', p=P)
    xc_c = xc[:].rearrange('(p t) d -> p (t d)', p=P)
    out_r = out[:].rearrange('(j p) d -> p j d', p=P)

    with tile.TileContext(nc) as tc:
        with tc.tile_pool(name='const', bufs=1) as cpool:
            beta_sb = cpool.tile([P, 1], f32)
            negb_sb = cpool.tile([P, 1], f32)
            nc.sync.dma_start(out=beta_sb[:], in_=beta_in[:])
            nc.vector.tensor_scalar_mul(out=negb_sb[:], in0=beta_sb[:], scalar1=-1.0)

            # ---- prologue: xh = x / sqrt(sum(x^2) + eps) into xt[:, 32:64]
            n_chunk = PRO_CHUNKS
            ct = -(-NT // n_chunk)
            with tc.tile_pool(name='pro', bufs=PRO_BUFS) as ppool:
                for c0 in range(0, NT, ct):
                    cw = min(ct, NT - c0)
                    xtile = ppool.tile([P, ct * D], f32, tag='xtile')
                    sq = ppool.tile([P, ct * D], f32, tag='sq')
                    s = ppool.tile([P, ct], f32, tag='s')
                    inv = ppool.tile([P, ct], f32, tag='inv')
                    xh = ppool.tile([P, ct * D], f32, tag='xh')
                    xv = xtile[:, :cw * D].rearrange('p (t d) -> p t d', d=D)
                    # packed x rows for partition p are contiguous in xc
                    nc.sync.dma_start(out=xv, in_=xc_c[:, c0 * D:(c0 + cw) * D])
                    nc.gpsimd.tensor_tensor(
                        out=sq[:, :cw * D].rearrange('p (t d) -> p t d', d=D),
                        in0=xv, in1=xv, op=Alu.mult)
                    nc.vector.tensor_reduce(
                        out=s[:, :cw],
                        in_=sq[:, :cw * D].rearrange('p (t d) -> p t d', d=D),
                        axis=mybir.AxisListType.X, op=Alu.add)
                    nc.vector.tensor_scalar_add(
                        out=s[:, :cw], in0=s[:, :cw], scalar1=1e-30)
                    nc.vector.reciprocal(out=inv[:, :cw], in_=s[:, :cw])
                    nc.scalar.activation(out=inv[:, :cw], in_=inv[:, :cw],
                                         func=Act.Sqrt)
                    xhv = xh[:, :cw * D].rearrange('p (t d) -> p t d', d=D)
                    nc.vector.tensor_tensor(
                        out=xhv,
                        in0=xv,
                        in1=inv[:, :cw].unsqueeze(2).broadcast_to([P, cw, D]),
                        op=Alu.mult)
                    nc.scalar.dma_start(
                        out=xt_c[:].rearrange('p (t e) -> p t e', e=ELEM)
                            [:, c0:c0 + cw, D:ELEM],
                        in_=xhv)

            # ---- main loop over rounds
            off_st = 0
            with tc.tile_pool(name='stp', bufs=4) as stpool, \
                    tc.tile_pool(name='main', bufs=2) as mpool:
                for j, (kl, kh) in enumerate(shapes):
                    own_hi = j >= n_lo_rounds
                    nil, nih = P * T * kl, P * T * kh
                    wi_l, wi_h = nil // 16, nih // 16
                    w_rnd = wi_l + wi_h + 2 * T * (kl + kh)
                    st = stpool.tile([P, w_rnd], i16, tag='st')
                    nc.sync.dma_start(out=st[:], in_=stream[:, off_st:off_st + w_rnd])
                    off_st += w_rnd
                    it_l = st[:, 0:wi_l]
                    it_h = st[:, wi_l:wi_l + wi_h]
                    o2 = wi_l + wi_h
                    bt_l = st[:, o2:o2 + 2 * T * kl].bitcast(f32)
                    bt_h = st[:, o2 + 2 * T * kl:o2 + 2 * T * (kl + kh)].bitcast(f32)

                    g_l = mpool.tile([P, T * kl * ELEM], f32, tag='g_l')
                    g_h = mpool.tile([P, T * kh * ELEM], f32, tag='g_h')
                    nc.gpsimd.dma_gather(
                        out_ap=g_l[:].rearrange('p (m e) -> p m e', e=ELEM),
                        in_ap=xt[0:half], idxs_ap=it_l,
                        num_idxs=nil, num_idxs_reg=nil, elem_size=ELEM,
                        single_packet=False)
                    nc.gpsimd.dma_gather(
                        out_ap=g_h[:].rearrange('p (m e) -> p m e', e=ELEM),
                        in_ap=xt[half:], idxs_ap=it_h,
                        num_idxs=nih, num_idxs_reg=nih, elem_size=ELEM,
                        single_packet=False)

                    gv_l = g_l[:].rearrange('p (t k e) -> p t k e', t=T, e=ELEM)
                    gv_h = g_h[:].rearrange('p (t k e) -> p t k e', t=T, e=ELEM)
                    gv_own = gv_h if own_hi else gv_l
                    k_own = kh if own_hi else kl

                    dm_dt = mybir.dt.bfloat16 if DM_BF16 else f32
                    dm_l = mpool.tile([P, T * kl * D], dm_dt, tag='dm_l')
                    dm_h = mpool.tile([P, T * kh * D], dm_dt, tag='dm_h')
                    a_l = mpool.tile([P, T * kl], f32, tag='a_l')
                    a_h = mpool.tile([P, T * kh], f32, tag='a_h')
                    e_l = mpool.tile([P, T * kl], f32, tag='e_l')
                    e_h = mpool.tile([P, T * kh], f32, tag='e_h')
                    den_l = mpool.tile([P, T], f32, tag='den_l')
                    den_h = mpool.tile([P, T], f32, tag='den_h')
                    rec = mpool.tile([P, T], f32, tag='rec')
                    p_l = mpool.tile([P, T * D * kl], dm_dt, tag='dm_l')
                    p_h = mpool.tile([P, T * D * kh], dm_dt, tag='dm_h')
                    ag_l = mpool.tile([P, T * D], f32, tag='ag_l')
                    ag_h = mpool.tile([P, T * D], f32, tag='ag_h')
                    osb = mpool.tile([P, T * D], f32, tag='osb')

                    for si, (gv, kk, dm, a, bt, ee, den) in enumerate((
                            (gv_l, kl, dm_l, a_l, bt_l, e_l, den_l),
                            (gv_h, kh, dm_h, a_h, bt_h, e_h, den_h))):
                        on_pool = f'dm{si}' in OFFLOAD_POOL and not (
                            si == 0 and (j % 5) < BALANCE_DM0_DVE)
                        mul_eng = nc.gpsimd if on_pool else nc.vector
                        self_xh = gv_own[:, :, 0:1, D:ELEM].broadcast_to(
                            [P, T, kk, D])
                        dmv = dm[:].rearrange('p (t k d) -> p t k d', t=T, d=D)
                        mul_eng.tensor_tensor(
                            out=dmv, in0=gv[:, :, :, D:ELEM], in1=self_xh,
                            op=Alu.mult)
                        nc.vector.tensor_reduce(
                            out=a[:].rearrange('p (t k) -> p t k', t=T),
                            in_=dmv, axis=mybir.AxisListType.X, op=Alu.add)
                        nc.vector.tensor_tensor(
                            out=a[:], in0=a[:], in1=bt, op=Alu.add)
                        for t in range(T):
                            nc.scalar.activation(
                                out=ee[:, t * kk:(t + 1) * kk],
                                in_=a[:, t * kk:(t + 1) * kk],
                                func=Act.Exp, bias=negb_sb[:], scale=beta_sb[:],
                                accum_out=den[:, t:t + 1])

                    nc.vector.tensor_tensor(out=rec[:], in0=den_l[:],
                                            in1=den_h[:], op=Alu.add)
                    nc.vector.reciprocal(out=rec[:], in_=rec[:])

                    for si, (gv, kk, ee, pp, ag) in enumerate((
                            (gv_l, kl, e_l, p_l, ag_l),
                            (gv_h, kh, e_h, p_h, ag_h))):
                        mul_eng = nc.gpsimd if f'p{si}' in OFFLOAD_POOL \
                            else nc.vector
                        ppv = pp[:].rearrange('p (t d k) -> p t d k', t=T, d=D)
                        mul_eng.tensor_tensor(
                            out=ppv,
                            in0=gv[:, :, :, 0:D].transpose([0, 1, 3, 2]),
                            in1=ee[:].rearrange('p (t k) -> p t k', t=T)
                                .unsqueeze(2).broadcast_to([P, T, D, kk]),
                            op=Alu.mult)
                        nc.vector.tensor_reduce(
                            out=ag[:].rearrange('p (t d) -> p t d', t=T),
                            in_=ppv, axis=mybir.AxisListType.X, op=Alu.add)

                    agv = ag_l[:].rearrange('p (t d) -> p t d', t=T)
                    nc.vector.tensor_tensor(
                        out=agv, in0=agv,
                        in1=ag_h[:].rearrange('p (t d) -> p t d', t=T), op=Alu.add)
                    nc.vector.tensor_tensor(
                        out=agv, in0=agv,
                        in1=rec[:].unsqueeze(2).broadcast_to([P, T, D]),
                        op=Alu.mult)
                    nc.vector.tensor_scalar_max(out=ag_l[:], in0=ag_l[:],
                                                scalar1=0.0)
                    nc.vector.tensor_tensor(
                        out=osb[:].rearrange('p (t d) -> p t d', t=T)
                            .unsqueeze(2),
                        in0=agv.unsqueeze(2),
                        in1=gv_own[:, :, 0:1, 0:D], op=Alu.add)
                    nc.sync.dma_start(
                        out=out_r[:, j * T:(j + 1) * T, :],
                        in_=osb[:].rearrange('p (t d) -> p t d', t=T))
    nc.finalize()
    return nc


def build_stream(pre, c):
    """Concatenate one core's per-round [idx_lo | idx_hi | b_lo | b_hi]
    planes into a single int16 stream (bias bitcast to 2x int16)."""
    cc = pre['cores'][c]
    parts = []
    off_il = off_ih = off_bl = off_bh = 0
    for j, (kl, kh) in enumerate(pre['shapes']):
        wi_l, wi_h = P * T * kl // 16, P * T * kh // 16
        parts.append(cc['idx_lo'][:, off_il:off_il + wi_l])
        parts.append(cc['idx_hi'][:, off_ih:off_ih + wi_h])
        parts.append(cc['b_lo'][:, off_bl:off_bl + T * kl].view(np.int16))
        parts.append(cc['b_hi'][:, off_bh:off_bh + T * kh].view(np.int16))
        off_il += wi_l; off_ih += wi_h
        off_bl += T * kl; off_bh += T * kh
    return np.ascontiguousarray(np.concatenate(parts, axis=1))


def kernel(x, edge_index, beta):
    from concourse.bass_utils import run_bass_kernel_spmd

    x = np.asarray(x, dtype=np.float32)
    edge_index = np.asarray(edge_index)
    beta = np.asarray(beta, dtype=np.float32)
    n_nodes = x.shape[0]

    pre = preprocess(edge_index, n_nodes)
    half, n_pad = pre['half'], pre['n_pad']
    shapes = pre['shapes']
    streams = [build_stream(pre, c) for c in range(N_CORES)]
    w_stream = streams[0].shape[1]

    key = (tuple(shapes), pre['n_lo_rounds'], half, n_pad, w_stream)
    nc = _PROG_CACHE.get(key)
    if nc is None:
        nc = build_program(shapes, pre['n_lo_rounds'], half, n_pad, w_stream)
        _PROG_CACHE[key] = nc

    xt = np.zeros((n_pad, ELEM), dtype=np.float32)
    xt[:n_nodes, :D] = x
    xc = np.zeros((n_pad, D), dtype=np.float32)
    xc[:n_nodes] = x
    beta_bc = np.tile(beta.reshape(1, 1), (P, 1)).astype(np.float32)

    in_maps = []
    for c in range(N_CORES):
        in_maps.append({'xt': xt, 'xc': xc, 'stream': streams[c],
                        'beta': beta_bc})
    res = run_bass_kernel_spmd(nc, in_maps, list(range(N_CORES)))

    out_full = np.zeros((n_nodes, D), dtype=np.float32)
    for c in range(N_CORES):
        cc = pre['cores'][c]
        o = res.results[c]['out']
        # device row (j*T + t)*128 + p holds node cc['nodes'][(j*T+t)*128 + p]?
        # nodes are stored per supertile in [t*128 + p] order already.
        m = cc['real']
        out_full[cc['nodes'][m]] = o[m]
    return out_full


if __name__ == '__main__':
    # quick self-check against a jax-free reference on small data
    rng = np.random.default_rng(1)
    n, e = 3000, 60000
    x = rng.standard_normal((n, D)).astype(np.float32)
    ei = rng.integers(0, n, size=(2, e)).astype(np.int64)
    beta = np.ones((1,), np.float32)

    def ref(x, edge_index, beta):
        N = x.shape[0]
        loops = np.arange(N, dtype=np.int64)
        s = np.concatenate([edge_index[0], loops])
        d = np.concatenate([edge_index[1], loops])
        nrm = np.sqrt((x * x).sum(-1, keepdims=True))
        xn = x / np.maximum(nrm, 1e-12)
        alpha = beta[0] * (xn[d] * xn[s]).sum(-1)
        amax = np.full(N, -np.inf)
        np.maximum.at(amax, d, alpha)
        ex = np.exp(alpha - amax[d])
        den = np.zeros(N)
        np.add.at(den, d, ex)
        w = ex / np.maximum(den[d], 1e-12)
        out = np.zeros_like(x)
        np.add.at(out, d, (w[:, None] * x[s]).astype(np.float32))
        return x + np.maximum(out, 0.0)

    want = ref(x, ei, beta)
    pre = preprocess(ei, n)
    got = emulate(x, beta, pre)
    err = np.abs(got - want).max() / np.abs(want).max()
    ks = np.array(pre['shapes'])
    tot_slots = (ks.sum(1) * P * T).sum() * N_CORES
    print('emulate rel err:', err)
    print('rounds:', len(pre['shapes']), 'slot amplification:',
          tot_slots / (e + n))


# revision 31
# speedup vs baseline: 1.1090x; 1.1090x over previous
"""AGNN conv kernel for trn2: out = x + relu(agnn_conv(x, edge_index, beta)).

Strategy: destination-sharded edge partitioning across 8 NeuronCores.
Host-side index preprocessing builds a padded CSR (incoming src lists per
node, incl. self loop) with nodes packed into degree-homogeneous tiles of
128 (one node per SBUF partition). Each core gathers [x | x_norm] rows of
its slot table from HBM via dma_gather (256B rows), computes the cosine
attention softmax densely per partition, and writes its output rows.
No cross-core communication is needed (all edges of a node live on one
core); the host re-assembles the full output.

The x_norm half of the gather table is computed on device in a prologue.
int16 gather indices only span 32k rows, so the table is split into lo/hi
halves (row < HALF vs >= HALF) and every node tile carries two slot
structures, one per half. Tiles are half-pure so the node's own (self)
row sits at slot 0 of its own half's structure.
"""

import sys
import numpy as np

sys.path.insert(0, '/opt/trn_rl_repo')

N_CORES = 8
P = 128           # SBUF partitions / nodes per tile
T = 2             # node tiles per supertile (one gather pair per supertile)
D = 32            # feature dim
ELEM = 64         # f32 per table row: [x(32) | xh(32)] = 256B
K_ROUND = 1       # round slot counts up to a multiple of this
PAD_BIAS = -1.0e9
OFFLOAD_POOL = {'dm0', 'dm1', 'p0', 'p1'}  # muls to run on GPSIMD
BALANCE_DM0_DVE = 4  # of every 5 rounds, run dm0 on DVE this many
BALANCE_P0_DVE = 0   # same for the lo P-mul
PRO_BUFS = 6
PRO_CHUNKS = 14
DM_BF16 = False        # write product tiles in bf16 (faster single-src reduce)


# ---------------------------------------------------------------------------
# host-side index preprocessing
# ---------------------------------------------------------------------------

def preprocess(edge_index, n_nodes):
    """Build per-core gather/bias planes and node lists.

    Returns a dict with everything the device program and the output
    unshuffle need. Only integer index manipulation happens here.
    """
    n_pad = -(-n_nodes // P) * P            # table rows padded to 128
    half = n_pad // 2                        # lo rows [0, half), hi [half, 2*half)
    src = np.asarray(edge_index[0], dtype=np.int64)
    dst = np.asarray(edge_index[1], dtype=np.int64)

    is_hi = src >= half
    # CSR of incoming edges per (dst, half): sort by dst with lo srcs first
    key = dst * 2 + is_hi
    order = np.argsort(key, kind='stable')
    src_sorted = src[order]
    d_lo = np.bincount(dst[~is_hi], minlength=n_nodes)
    d_hi = np.bincount(dst[is_hi], minlength=n_nodes)
    deg = d_lo + d_hi
    # starts[i] = first edge of node i in src_sorted; lo edges then hi edges
    starts = np.zeros(n_nodes + 1, dtype=np.int64)
    np.cumsum(deg, out=starts[1:])

    node_is_hi = np.arange(n_nodes) >= half
    slots_lo = d_lo + (~node_is_hi)          # self slot for lo nodes
    slots_hi = d_hi + node_is_hi

    sup_nodes = P * T

    def build_half(ids_real):
        """Order one half's nodes into padded supertiles; return node array.

        Sort key buckets d_lo coarsely then orders by d_hi so that both
        per-tile maxima stay close to the per-node values (minimises padded
        slots; a plain total-degree sort leaves the lo/hi binomial split
        unpacked and costs ~15% more gather traffic).
        """
        a = slots_lo[ids_real].astype(np.int64)
        b = slots_hi[ids_real].astype(np.int64)
        key = np.maximum(a, b) * 100000 + np.minimum(a, b)
        ids_sorted = ids_real[np.argsort(key, kind='stable')]
        n_sup = -(-len(ids_sorted) // sup_nodes)
        n_sup = -(-n_sup // N_CORES) * N_CORES          # rounds of 8
        padded = np.full(n_sup * sup_nodes, ids_sorted[0], dtype=np.int64)
        padded[:len(ids_sorted)] = ids_sorted
        real = np.zeros(n_sup * sup_nodes, dtype=bool)
        real[:len(ids_sorted)] = True
        return padded.reshape(n_sup, sup_nodes), real.reshape(n_sup, sup_nodes)

    lo_ids = np.arange(0, min(half, n_nodes))
    hi_ids = np.arange(half, n_nodes)
    sup_lo, real_lo = build_half(lo_ids)
    sup_hi, real_hi = build_half(hi_ids)

    # deal supertiles (sorted by cost desc) round-robin to cores;
    # every core's round j shares the max K of the 8 supertiles in it.
    def deal(sup, real):
        cost = np.array([
            max(slots_lo[s].max(), slots_hi[s].max()) for s in sup])
        o = np.argsort(-cost, kind='stable')
        sup, real = sup[o], real[o]
        n_rounds = len(sup) // N_CORES
        rounds = []
        for j in range(n_rounds):
            grp = sup[j * N_CORES:(j + 1) * N_CORES]
            grp_real = real[j * N_CORES:(j + 1) * N_CORES]
            kl = int(max(slots_lo[g].max() for g in grp))
            kh = int(max(slots_hi[g].max() for g in grp))
            kl = max(1, -(-kl // K_ROUND) * K_ROUND)
            kh = max(1, -(-kh // K_ROUND) * K_ROUND)
            rounds.append((grp, grp_real, kl, kh))
        return rounds

    rounds = deal(sup_lo, real_lo) + deal(sup_hi, real_hi)
    n_lo_rounds = sup_lo.shape[0] // N_CORES

    col = None  # lazily sized scratch

    def slot_matrix(ids, own_half_is_hi, want_hi):
        """[len(ids), K] int16 slot matrix + bias for one structure."""
        nonlocal col
        n = len(ids)
        base = half if want_hi else 0
        if want_hi:
            cnt_edges = d_hi[ids]
            edge_start = starts[ids] + d_lo[ids]
        else:
            cnt_edges = d_lo[ids]
            edge_start = starts[ids]
        own = own_half_is_hi == want_hi
        self_off = 1 if own else 0
        cnt = cnt_edges + self_off
        K = int(cnt.max())
        K = max(1, -(-K // K_ROUND) * K_ROUND)
        S = np.zeros((n, K), dtype=np.int64)
        cols = np.arange(K)[None, :]
        valid = cols < cnt[:, None]
        if own:
            S[:, 0] = ids - base
            e_col = cols - 1
        else:
            e_col = cols
        take = edge_start[:, None] + e_col
        e_valid = valid & (e_col >= 0)
        S[e_valid] = src_sorted[np.clip(take, 0, len(src_sorted) - 1)][e_valid] - base
        bias = np.where(valid, 0.0, PAD_BIAS).astype(np.float32)
        return S.astype(np.int16), bias, K

    # per-core streams
    cores = [{'idx_lo': [], 'idx_hi': [], 'b_lo': [], 'b_hi': [],
              'nodes': [], 'real': []} for _ in range(N_CORES)]
    shapes = []  # (kl, kh) per round, shared across cores

    def wrap16(L):
        # unwrapped[j] = plane[j % 16, j // 16]; replicate over 8 groups
        plane = L.reshape(-1, 16).T.copy()
        return np.tile(plane, (8, 1))

    for j, (grp, grp_real, kl, kh) in enumerate(rounds):
        own_hi = j >= n_lo_rounds
        shapes.append((kl, kh))
        for c in range(N_CORES):
            ids = grp[c]
            S_lo, B_lo, _ = pad_to(slot_matrix(ids, own_hi, False), kl)
            S_hi, B_hi, _ = pad_to(slot_matrix(ids, own_hi, True), kh)
            # index order j = g*128 + p with g = t*K + k
            L_lo = S_lo.reshape(T, P, kl).transpose(0, 2, 1).reshape(-1)
            L_hi = S_hi.reshape(T, P, kh).transpose(0, 2, 1).reshape(-1)
            cores[c]['idx_lo'].append(wrap16(L_lo))
            cores[c]['idx_hi'].append(wrap16(L_hi))
            # bias planes [128, T*K] in [p, t*K + k] layout
            cores[c]['b_lo'].append(
                B_lo.reshape(T, P, kl).transpose(1, 0, 2).reshape(P, T * kl))
            cores[c]['b_hi'].append(
                B_hi.reshape(T, P, kh).transpose(1, 0, 2).reshape(P, T * kh))
            cores[c]['nodes'].append(grp[c])
            cores[c]['real'].append(grp_real[c])

    for c in range(N_CORES):
        cc = cores[c]
        cc['idx_lo'] = np.concatenate(cc['idx_lo'], axis=1)
        cc['idx_hi'] = np.concatenate(cc['idx_hi'], axis=1)
        cc['b_lo'] = np.concatenate(cc['b_lo'], axis=1)
        cc['b_hi'] = np.concatenate(cc['b_hi'], axis=1)
        cc['nodes'] = np.concatenate(cc['nodes'])
        cc['real'] = np.concatenate(cc['real'])

    return {
        'cores': cores, 'shapes': shapes, 'n_lo_rounds': n_lo_rounds,
        'half': half, 'n_pad': n_pad,
    }


def pad_to(smb, K):
    """Pad a (S, bias, k) triple's columns out to K."""
    S, B, k = smb
    if k == K:
        return S, B, K
    assert k < K
    n = S.shape[0]
    S2 = np.zeros((n, K), dtype=np.int16)
    S2[:, :k] = S
    B2 = np.full((n, K), PAD_BIAS, dtype=np.float32)
    B2[:, :k] = B
    return S2, B2, K


# ---------------------------------------------------------------------------
# numpy emulation of the device program (for validation)
# ---------------------------------------------------------------------------

def emulate(x, beta, pre):
    n_nodes = x.shape[0]
    half, n_pad = pre['half'], pre['n_pad']
    xt = np.zeros((n_pad, ELEM), dtype=np.float32)
    xt[:n_nodes, :D] = x
    # device prologue: xh = x * 1/sqrt(sum(x^2) + 1e-30)
    s = (xt[:, :D] ** 2).sum(-1) + 1e-30
    inv_n = np.sqrt((1.0 / s)).astype(np.float32)
    xt[:, D:] = xt[:, :D] * inv_n[:, None]

    b = float(beta[0])
    out_full = np.zeros((n_nodes, D), dtype=np.float32)
    shapes = pre['shapes']
    n_lo_rounds = pre['n_lo_rounds']

    for c in range(N_CORES):
        cc = pre['cores'][c]
        off_il = off_ih = off_bl = off_bh = 0
        outs = []
        for j, (kl, kh) in enumerate(shapes):
            own_hi = j >= n_lo_rounds
            nil, nih = P * T * kl, P * T * kh
            plane_l = cc['idx_lo'][:16, off_il:off_il + nil // 16]
            plane_h = cc['idx_hi'][:16, off_ih:off_ih + nih // 16]
            off_il += nil // 16; off_ih += nih // 16
            L_lo = plane_l.T.reshape(-1)
            L_hi = plane_h.T.reshape(-1)
            B_lo = cc['b_lo'][:, off_bl:off_bl + T * kl]; off_bl += T * kl
            B_hi = cc['b_hi'][:, off_bh:off_bh + T * kh]; off_bh += T * kh
            # gather: G[p, g, :] = slice[L[g*128+p]]
            G_lo = xt[:half][L_lo.reshape(T * kl, P).T.astype(np.int64)]
            G_hi = xt[half:][L_hi.reshape(T * kh, P).T.astype(np.int64)]
            G_lo = G_lo.reshape(P, T, kl, ELEM)
            G_hi = G_hi.reshape(P, T, kh, ELEM)
            G_own = G_hi if own_hi else G_lo
            xh_self = G_own[:, :, 0, D:]                      # [P, T, 32]
            x_self = G_own[:, :, 0, :D]
            dot_lo = (G_lo[:, :, :, D:] * xh_self[:, :, None, :]).sum(-1)
            dot_hi = (G_hi[:, :, :, D:] * xh_self[:, :, None, :]).sum(-1)
            a_lo = dot_lo + B_lo.reshape(P, T, kl)
            a_hi = dot_hi + B_hi.reshape(P, T, kh)
            e_lo = np.exp(b * a_lo - b)
            e_hi = np.exp(b * a_hi - b)
            den = e_lo.sum(-1) + e_hi.sum(-1)                 # [P, T]
            agg = (e_lo[..., None] * G_lo[:, :, :, :D]).sum(2) \
                + (e_hi[..., None] * G_hi[:, :, :, :D]).sum(2)
            o = x_self + np.maximum(agg / den[..., None], 0.0)
            outs.append(o.transpose(1, 0, 2).reshape(T * P, D))
        out_c = np.concatenate(outs, 0)
        m = cc['real']
        out_full[cc['nodes'][m]] = out_c[m]
    return out_full


# ---------------------------------------------------------------------------
# device program
# ---------------------------------------------------------------------------

_PROG_CACHE = {}


def build_program(shapes, n_lo_rounds, half, n_pad, w_stream):
    import concourse.bass as bass
    import concourse.bacc as bacc
    import concourse.tile as tile
    from concourse import mybir

    f32 = mybir.dt.float32
    i16 = mybir.dt.int16
    Alu = mybir.AluOpType
    Act = mybir.ActivationFunctionType
    n_rounds = len(shapes)
    n_out = n_rounds * T * P

    nc = bacc.Bacc()
    xt = nc.declare_dram_parameter('xt', [n_pad, ELEM], f32, isOutput=False)
    xc = nc.declare_dram_parameter('xc', [n_pad, D], f32, isOutput=False)
    stream = nc.declare_dram_parameter('stream', [P, w_stream], i16, isOutput=False)
    beta_in = nc.declare_dram_parameter('beta', [P, 1], f32, isOutput=False)
    out = nc.declare_dram_parameter('out', [n_out, D], f32, isOutput=True)

    NT = n_pad // P                            # t-cols per partition (e.g. 391)
    xt_c = xt[:].rearrange('(p t) e -> p (t e)', p=P)
    xc_c = xc[:].rearrange('(p t) d -> p (t d)', p=P)
    out_r = out[:].rearrange('(j p) d -> p j d', p=P)

    with tile.TileContext(nc) as tc:
        with tc.tile_pool(name='const', bufs=1) as cpool:
            beta_sb = cpool.tile([P, 1], f32)
            negb_sb = cpool.tile([P, 1], f32)
            nc.sync.dma_start(out=beta_sb[:], in_=beta_in[:])
            nc.vector.tensor_scalar_mul(out=negb_sb[:], in0=beta_sb[:], scalar1=-1.0)

            # ---- prologue: xh = x / sqrt(sum(x^2) + eps) into xt[:, 32:64]
            n_chunk = PRO_CHUNKS
            ct = -(-NT // n_chunk)
            with tc.tile_pool(name='pro', bufs=PRO_BUFS) as ppool:
                for c0 in range(0, NT, ct):
                    cw = min(ct, NT - c0)
                    xtile = ppool.tile([P, ct * D], f32, tag='xtile')
                    sq = ppool.tile([P, ct * D], f32, tag='sq')
                    s = ppool.tile([P, ct], f32, tag='s')
                    inv = ppool.tile([P, ct], f32, tag='inv')
                    xh = ppool.tile([P, ct * D], f32, tag='xh')
                    xv = xtile[:, :cw * D].rearrange('p (t d) -> p t d', d=D)
                    # packed x rows for partition p are contiguous in xc
                    nc.sync.dma_start(out=xv, in_=xc_c[:, c0 * D:(c0 + cw) * D])
                    nc.gpsimd.tensor_tensor(
                        out=sq[:, :cw * D].rearrange('p (t d) -> p t d', d=D),
                        in0=xv, in1=xv, op=Alu.mult)
                    nc.vector.tensor_reduce(
                        out=s[:, :cw],
                        in_=sq[:, :cw * D].rearrange('p (t d) -> p t d', d=D),
                        axis=mybir.AxisListType.X, op=Alu.add)
                    nc.vector.tensor_scalar_add(
                        out=s[:, :cw], in0=s[:, :cw], scalar1=1e-30)
                    nc.vector.reciprocal(out=inv[:, :cw], in_=s[:, :cw])
                    nc.scalar.activation(out=inv[:, :cw], in_=inv[:, :cw],
                                         func=Act.Sqrt)
                    xhv = xh[:, :cw * D].rearrange('p (t d) -> p t d', d=D)
                    nc.vector.tensor_tensor(
                        out=xhv,
                        in0=xv,
                        in1=inv[:, :cw].unsqueeze(2).broadcast_to([P, cw, D]),
                        op=Alu.mult)
                    # alternate the two HWDGE rings (SP / ACT) so the
                    # strided table writes don't serialize on one ring
                    weng = nc.scalar if (c0 // ct) % 2 else nc.sync
                    weng.dma_start(
                        out=xt_c[:].rearrange('p (t e) -> p t e', e=ELEM)
                            [:, c0:c0 + cw, D:ELEM],
                        in_=xhv)

            # ---- main loop over rounds
            off_st = 0
            with tc.tile_pool(name='stp', bufs=4) as stpool, \
                    tc.tile_pool(name='main', bufs=3) as mpool:
                for j, (kl, kh) in enumerate(shapes):
                    own_hi = j >= n_lo_rounds
                    nil, nih = P * T * kl, P * T * kh
                    wi_l, wi_h = nil // 16, nih // 16
                    w_rnd = wi_l + wi_h + 2 * T * (kl + kh)
                    st = stpool.tile([P, w_rnd], i16, tag='st')
                    nc.sync.dma_start(out=st[:], in_=stream[:, off_st:off_st + w_rnd])
                    off_st += w_rnd
                    it_l = st[:, 0:wi_l]
                    it_h = st[:, wi_l:wi_l + wi_h]
                    o2 = wi_l + wi_h
                    bt_l = st[:, o2:o2 + 2 * T * kl].bitcast(f32)
                    bt_h = st[:, o2 + 2 * T * kl:o2 + 2 * T * (kl + kh)].bitcast(f32)

                    g_l = mpool.tile([P, T * kl * ELEM], f32, tag='g_l')
                    g_h = mpool.tile([P, T * kh * ELEM], f32, tag='g_h')
                    nc.gpsimd.dma_gather(
                        out_ap=g_l[:].rearrange('p (m e) -> p m e', e=ELEM),
                        in_ap=xt[0:half], idxs_ap=it_l,
                        num_idxs=nil, num_idxs_reg=nil, elem_size=ELEM,
                        single_packet=False)
                    nc.gpsimd.dma_gather(
                        out_ap=g_h[:].rearrange('p (m e) -> p m e', e=ELEM),
                        in_ap=xt[half:], idxs_ap=it_h,
                        num_idxs=nih, num_idxs_reg=nih, elem_size=ELEM,
                        single_packet=False)

                    gv_l = g_l[:].rearrange('p (t k e) -> p t k e', t=T, e=ELEM)
                    gv_h = g_h[:].rearrange('p (t k e) -> p t k e', t=T, e=ELEM)
                    gv_own = gv_h if own_hi else gv_l
                    k_own = kh if own_hi else kl

                    dm_dt = mybir.dt.bfloat16 if DM_BF16 else f32
                    dm_l = mpool.tile([P, T * kl * D], dm_dt, tag='dm_l')
                    dm_h = mpool.tile([P, T * kh * D], dm_dt, tag='dm_h')
                    a_l = mpool.tile([P, T * kl], f32, tag='a_l')
                    a_h = mpool.tile([P, T * kh], f32, tag='a_h')
                    e_l = mpool.tile([P, T * kl], f32, tag='e_l')
                    e_h = mpool.tile([P, T * kh], f32, tag='e_h')
                    den_l = mpool.tile([P, T], f32, tag='den_l')
                    den_h = mpool.tile([P, T], f32, tag='den_h')
                    rec = mpool.tile([P, T], f32, tag='rec')
                    p_l = mpool.tile([P, T * D * kl], dm_dt, tag='dm_l')
                    p_h = mpool.tile([P, T * D * kh], dm_dt, tag='dm_h')
                    ag_l = mpool.tile([P, T * D], f32, tag='ag_l')
                    ag_h = mpool.tile([P, T * D], f32, tag='ag_h')
                    osb = mpool.tile([P, T * D], f32, tag='osb')

                    for si, (gv, kk, dm, a, bt, ee, den) in enumerate((
                            (gv_l, kl, dm_l, a_l, bt_l, e_l, den_l),
                            (gv_h, kh, dm_h, a_h, bt_h, e_h, den_h))):
                        on_pool = f'dm{si}' in OFFLOAD_POOL and not (
                            si == 0 and (j % 5) < BALANCE_DM0_DVE)
                        mul_eng = nc.gpsimd if on_pool else nc.vector
                        self_xh = gv_own[:, :, 0:1, D:ELEM].broadcast_to(
                            [P, T, kk, D])
                        dmv = dm[:].rearrange('p (t k d) -> p t k d', t=T, d=D)
                        mul_eng.tensor_tensor(
                            out=dmv, in0=gv[:, :, :, D:ELEM], in1=self_xh,
                            op=Alu.mult)
                        nc.vector.tensor_reduce(
                            out=a[:].rearrange('p (t k) -> p t k', t=T),
                            in_=dmv, axis=mybir.AxisListType.X, op=Alu.add)
                        nc.vector.tensor_tensor(
                            out=a[:], in0=a[:], in1=bt, op=Alu.add)
                        for t in range(T):
                            nc.scalar.activation(
                                out=ee[:, t * kk:(t + 1) * kk],
                                in_=a[:, t * kk:(t + 1) * kk],
                                func=Act.Exp, bias=negb_sb[:], scale=beta_sb[:],
                                accum_out=den[:, t:t + 1])

                    nc.vector.tensor_tensor(out=rec[:], in0=den_l[:],
                                            in1=den_h[:], op=Alu.add)
                    nc.vector.reciprocal(out=rec[:], in_=rec[:])

                    for si, (gv, kk, ee, pp, ag) in enumerate((
                            (gv_l, kl, e_l, p_l, ag_l),
                            (gv_h, kh, e_h, p_h, ag_h))):
                        mul_eng = nc.gpsimd if f'p{si}' in OFFLOAD_POOL \
                            else nc.vector
                        ppv = pp[:].rearrange('p (t d k) -> p t d k', t=T, d=D)
                        mul_eng.tensor_tensor(
                            out=ppv,
                            in0=gv[:, :, :, 0:D].transpose([0, 1, 3, 2]),
                            in1=ee[:].rearrange('p (t k) -> p t k', t=T)
                                .unsqueeze(2).broadcast_to([P, T, D, kk]),
                            op=Alu.mult)
                        nc.vector.tensor_reduce(
                            out=ag[:].rearrange('p (t d) -> p t d', t=T),
                            in_=ppv, axis=mybir.AxisListType.X, op=Alu.add)

                    agv = ag_l[:].rearrange('p (t d) -> p t d', t=T)
                    nc.vector.tensor_tensor(
                        out=agv, in0=agv,
                        in1=ag_h[:].rearrange('p (t d) -> p t d', t=T), op=Alu.add)
                    nc.vector.tensor_tensor(
                        out=agv, in0=agv,
                        in1=rec[:].unsqueeze(2).broadcast_to([P, T, D]),
                        op=Alu.mult)
                    nc.vector.tensor_scalar_max(out=ag_l[:], in0=ag_l[:],
                                                scalar1=0.0)
                    nc.vector.tensor_tensor(
                        out=osb[:].rearrange('p (t d) -> p t d', t=T)
                            .unsqueeze(2),
                        in0=agv.unsqueeze(2),
                        in1=gv_own[:, :, 0:1, 0:D], op=Alu.add)
                    nc.sync.dma_start(
                        out=out_r[:, j * T:(j + 1) * T, :],
                        in_=osb[:].rearrange('p (t d) -> p t d', t=T))
    nc.finalize()
    return nc


def build_stream(pre, c):
    """Concatenate one core's per-round [idx_lo | idx_hi | b_lo | b_hi]
    planes into a single int16 stream (bias bitcast to 2x int16)."""
    cc = pre['cores'][c]
    parts = []
    off_il = off_ih = off_bl = off_bh = 0
    for j, (kl, kh) in enumerate(pre['shapes']):
        wi_l, wi_h = P * T * kl // 16, P * T * kh // 16
        parts.append(cc['idx_lo'][:, off_il:off_il + wi_l])
        parts.append(cc['idx_hi'][:, off_ih:off_ih + wi_h])
        parts.append(cc['b_lo'][:, off_bl:off_bl + T * kl].view(np.int16))
        parts.append(cc['b_hi'][:, off_bh:off_bh + T * kh].view(np.int16))
        off_il += wi_l; off_ih += wi_h
        off_bl += T * kl; off_bh += T * kh
    return np.ascontiguousarray(np.concatenate(parts, axis=1))


def kernel(x, edge_index, beta):
    from concourse.bass_utils import run_bass_kernel_spmd

    x = np.asarray(x, dtype=np.float32)
    edge_index = np.asarray(edge_index)
    beta = np.asarray(beta, dtype=np.float32)
    n_nodes = x.shape[0]

    pre = preprocess(edge_index, n_nodes)
    half, n_pad = pre['half'], pre['n_pad']
    shapes = pre['shapes']
    streams = [build_stream(pre, c) for c in range(N_CORES)]
    w_stream = streams[0].shape[1]

    key = (tuple(shapes), pre['n_lo_rounds'], half, n_pad, w_stream)
    nc = _PROG_CACHE.get(key)
    if nc is None:
        nc = build_program(shapes, pre['n_lo_rounds'], half, n_pad, w_stream)
        _PROG_CACHE[key] = nc

    xt = np.zeros((n_pad, ELEM), dtype=np.float32)
    xt[:n_nodes, :D] = x
    xc = np.zeros((n_pad, D), dtype=np.float32)
    xc[:n_nodes] = x
    beta_bc = np.tile(beta.reshape(1, 1), (P, 1)).astype(np.float32)

    in_maps = []
    for c in range(N_CORES):
        in_maps.append({'xt': xt, 'xc': xc, 'stream': streams[c],
                        'beta': beta_bc})
    res = run_bass_kernel_spmd(nc, in_maps, list(range(N_CORES)))

    out_full = np.zeros((n_nodes, D), dtype=np.float32)
    for c in range(N_CORES):
        cc = pre['cores'][c]
        o = res.results[c]['out']
        # device row (j*T + t)*128 + p holds node cc['nodes'][(j*T+t)*128 + p]?
        # nodes are stored per supertile in [t*128 + p] order already.
        m = cc['real']
        out_full[cc['nodes'][m]] = o[m]
    return out_full


if __name__ == '__main__':
    # quick self-check against a jax-free reference on small data
    rng = np.random.default_rng(1)
    n, e = 3000, 60000
    x = rng.standard_normal((n, D)).astype(np.float32)
    ei = rng.integers(0, n, size=(2, e)).astype(np.int64)
    beta = np.ones((1,), np.float32)

    def ref(x, edge_index, beta):
        N = x.shape[0]
        loops = np.arange(N, dtype=np.int64)
        s = np.concatenate([edge_index[0], loops])
        d = np.concatenate([edge_index[1], loops])
        nrm = np.sqrt((x * x).sum(-1, keepdims=True))
        xn = x / np.maximum(nrm, 1e-12)
        alpha = beta[0] * (xn[d] * xn[s]).sum(-1)
        amax = np.full(N, -np.inf)
        np.maximum.at(amax, d, alpha)
        ex = np.exp(alpha - amax[d])
        den = np.zeros(N)
        np.add.at(den, d, ex)
        w = ex / np.maximum(den[d], 1e-12)
        out = np.zeros_like(x)
        np.add.at(out, d, (w[:, None] * x[s]).astype(np.float32))
        return x + np.maximum(out, 0.0)

    want = ref(x, ei, beta)
    pre = preprocess(ei, n)
    got = emulate(x, beta, pre)
    err = np.abs(got - want).max() / np.abs(want).max()
    ks = np.array(pre['shapes'])
    tot_slots = (ks.sum(1) * P * T).sum() * N_CORES
    print('emulate rel err:', err)
    print('rounds:', len(pre['shapes']), 'slot amplification:',
          tot_slots / (e + n))


# revision 35
# speedup vs baseline: 1.1256x; 1.0150x over previous
"""AGNN conv kernel for trn2: out = x + relu(agnn_conv(x, edge_index, beta)).

Strategy: destination-sharded edge partitioning across 8 NeuronCores.
Host-side index preprocessing builds a padded CSR (incoming src lists per
node, incl. self loop) with nodes packed into degree-homogeneous tiles of
128 (one node per SBUF partition). Each core gathers [x | x_norm] rows of
its slot table from HBM via dma_gather (256B rows), computes the cosine
attention softmax densely per partition, and writes its output rows.
No cross-core communication is needed (all edges of a node live on one
core); the host re-assembles the full output.

The x_norm half of the gather table is computed on device in a prologue.
int16 gather indices only span 32k rows, so the table is split into lo/hi
halves (row < HALF vs >= HALF) and every node tile carries two slot
structures, one per half. Tiles are half-pure so the node's own (self)
row sits at slot 0 of its own half's structure.
"""

import sys
import numpy as np

sys.path.insert(0, '/opt/trn_rl_repo')

N_CORES = 8
P = 128           # SBUF partitions / nodes per tile
T = 2             # node tiles per supertile (one gather pair per supertile)
D = 32            # feature dim
ELEM = 64         # f32 per table row: [x(32) | xh(32)] = 256B
K_ROUND = 1       # round slot counts up to a multiple of this
PAD_BIAS = -1.0e9
OFFLOAD_POOL = {'dm0', 'dm1', 'p0', 'p1'}  # muls to run on GPSIMD
BALANCE_DM0_DVE = 4  # of every 5 rounds, run dm0 on DVE this many
BALANCE_P0_DVE = 0   # same for the lo P-mul
PRO_BUFS = 6
PRO_CHUNKS = 10
DM_BF16 = False        # write product tiles in bf16 (faster single-src reduce)


# ---------------------------------------------------------------------------
# host-side index preprocessing
# ---------------------------------------------------------------------------

def preprocess(edge_index, n_nodes):
    """Build per-core gather/bias planes and node lists.

    Returns a dict with everything the device program and the output
    unshuffle need. Only integer index manipulation happens here.
    """
    n_pad = -(-n_nodes // P) * P            # table rows padded to 128
    half = n_pad // 2                        # lo rows [0, half), hi [half, 2*half)
    src = np.asarray(edge_index[0], dtype=np.int64)
    dst = np.asarray(edge_index[1], dtype=np.int64)

    is_hi = src >= half
    # CSR of incoming edges per (dst, half): sort by dst with lo srcs first
    key = dst * 2 + is_hi
    order = np.argsort(key, kind='stable')
    src_sorted = src[order]
    d_lo = np.bincount(dst[~is_hi], minlength=n_nodes)
    d_hi = np.bincount(dst[is_hi], minlength=n_nodes)
    deg = d_lo + d_hi
    # starts[i] = first edge of node i in src_sorted; lo edges then hi edges
    starts = np.zeros(n_nodes + 1, dtype=np.int64)
    np.cumsum(deg, out=starts[1:])

    node_is_hi = np.arange(n_nodes) >= half
    slots_lo = d_lo + (~node_is_hi)          # self slot for lo nodes
    slots_hi = d_hi + node_is_hi

    sup_nodes = P * T

    def build_half(ids_real):
        """Order one half's nodes into padded supertiles; return node array.

        Sort key buckets d_lo coarsely then orders by d_hi so that both
        per-tile maxima stay close to the per-node values (minimises padded
        slots; a plain total-degree sort leaves the lo/hi binomial split
        unpacked and costs ~15% more gather traffic).
        """
        a = slots_lo[ids_real].astype(np.int64)
        b = slots_hi[ids_real].astype(np.int64)
        key = np.maximum(a, b) * 100000 + np.minimum(a, b)
        ids_sorted = ids_real[np.argsort(key, kind='stable')]
        n_sup = -(-len(ids_sorted) // sup_nodes)
        n_sup = -(-n_sup // N_CORES) * N_CORES          # rounds of 8
        padded = np.full(n_sup * sup_nodes, ids_sorted[0], dtype=np.int64)
        padded[:len(ids_sorted)] = ids_sorted
        real = np.zeros(n_sup * sup_nodes, dtype=bool)
        real[:len(ids_sorted)] = True
        return padded.reshape(n_sup, sup_nodes), real.reshape(n_sup, sup_nodes)

    lo_ids = np.arange(0, min(half, n_nodes))
    hi_ids = np.arange(half, n_nodes)
    sup_lo, real_lo = build_half(lo_ids)
    sup_hi, real_hi = build_half(hi_ids)

    # deal supertiles (sorted by cost desc) round-robin to cores;
    # every core's round j shares the max K of the 8 supertiles in it.
    def deal(sup, real):
        cost = np.array([
            max(slots_lo[s].max(), slots_hi[s].max()) for s in sup])
        o = np.argsort(-cost, kind='stable')
        sup, real = sup[o], real[o]
        n_rounds = len(sup) // N_CORES
        rounds = []
        for j in range(n_rounds):
            grp = sup[j * N_CORES:(j + 1) * N_CORES]
            grp_real = real[j * N_CORES:(j + 1) * N_CORES]
            kl = int(max(slots_lo[g].max() for g in grp))
            kh = int(max(slots_hi[g].max() for g in grp))
            kl = max(1, -(-kl // K_ROUND) * K_ROUND)
            kh = max(1, -(-kh // K_ROUND) * K_ROUND)
            rounds.append((grp, grp_real, kl, kh))
        return rounds

    rounds = deal(sup_lo, real_lo) + deal(sup_hi, real_hi)
    n_lo_rounds = sup_lo.shape[0] // N_CORES

    col = None  # lazily sized scratch

    def slot_matrix(ids, own_half_is_hi, want_hi):
        """[len(ids), K] int16 slot matrix + bias for one structure."""
        nonlocal col
        n = len(ids)
        base = half if want_hi else 0
        if want_hi:
            cnt_edges = d_hi[ids]
            edge_start = starts[ids] + d_lo[ids]
        else:
            cnt_edges = d_lo[ids]
            edge_start = starts[ids]
        own = own_half_is_hi == want_hi
        self_off = 1 if own else 0
        cnt = cnt_edges + self_off
        K = int(cnt.max())
        K = max(1, -(-K // K_ROUND) * K_ROUND)
        S = np.zeros((n, K), dtype=np.int64)
        cols = np.arange(K)[None, :]
        valid = cols < cnt[:, None]
        if own:
            S[:, 0] = ids - base
            e_col = cols - 1
        else:
            e_col = cols
        take = edge_start[:, None] + e_col
        e_valid = valid & (e_col >= 0)
        S[e_valid] = src_sorted[np.clip(take, 0, len(src_sorted) - 1)][e_valid] - base
        bias = np.where(valid, 0.0, PAD_BIAS).astype(np.float32)
        return S.astype(np.int16), bias, K

    # per-core streams
    cores = [{'idx_lo': [], 'idx_hi': [], 'b_lo': [], 'b_hi': [],
              'nodes': [], 'real': []} for _ in range(N_CORES)]
    shapes = []  # (kl, kh) per round, shared across cores

    def wrap16(L):
        # unwrapped[j] = plane[j % 16, j // 16]; replicate over 8 groups
        plane = L.reshape(-1, 16).T.copy()
        return np.tile(plane, (8, 1))

    for j, (grp, grp_real, kl, kh) in enumerate(rounds):
        own_hi = j >= n_lo_rounds
        shapes.append((kl, kh))
        for c in range(N_CORES):
            ids = grp[c]
            S_lo, B_lo, _ = pad_to(slot_matrix(ids, own_hi, False), kl)
            S_hi, B_hi, _ = pad_to(slot_matrix(ids, own_hi, True), kh)
            # index order j = g*128 + p with g = t*K + k
            L_lo = S_lo.reshape(T, P, kl).transpose(0, 2, 1).reshape(-1)
            L_hi = S_hi.reshape(T, P, kh).transpose(0, 2, 1).reshape(-1)
            cores[c]['idx_lo'].append(wrap16(L_lo))
            cores[c]['idx_hi'].append(wrap16(L_hi))
            # bias planes [128, T*K] in [p, t*K + k] layout
            cores[c]['b_lo'].append(
                B_lo.reshape(T, P, kl).transpose(1, 0, 2).reshape(P, T * kl))
            cores[c]['b_hi'].append(
                B_hi.reshape(T, P, kh).transpose(1, 0, 2).reshape(P, T * kh))
            cores[c]['nodes'].append(grp[c])
            cores[c]['real'].append(grp_real[c])

    for c in range(N_CORES):
        cc = cores[c]
        cc['idx_lo'] = np.concatenate(cc['idx_lo'], axis=1)
        cc['idx_hi'] = np.concatenate(cc['idx_hi'], axis=1)
        cc['b_lo'] = np.concatenate(cc['b_lo'], axis=1)
        cc['b_hi'] = np.concatenate(cc['b_hi'], axis=1)
        cc['nodes'] = np.concatenate(cc['nodes'])
        cc['real'] = np.concatenate(cc['real'])

    return {
        'cores': cores, 'shapes': shapes, 'n_lo_rounds': n_lo_rounds,
        'half': half, 'n_pad': n_pad,
    }


def pad_to(smb, K):
    """Pad a (S, bias, k) triple's columns out to K."""
    S, B, k = smb
    if k == K:
        return S, B, K
    assert k < K
    n = S.shape[0]
    S2 = np.zeros((n, K), dtype=np.int16)
    S2[:, :k] = S
    B2 = np.full((n, K), PAD_BIAS, dtype=np.float32)
    B2[:, :k] = B
    return S2, B2, K


# ---------------------------------------------------------------------------
# numpy emulation of the device program (for validation)
# ---------------------------------------------------------------------------

def emulate(x, beta, pre):
    n_nodes = x.shape[0]
    half, n_pad = pre['half'], pre['n_pad']
    xt = np.zeros((n_pad, ELEM), dtype=np.float32)
    xt[:n_nodes, :D] = x
    # device prologue: xh = x * 1/sqrt(sum(x^2) + 1e-30)
    s = (xt[:, :D] ** 2).sum(-1) + 1e-30
    inv_n = np.sqrt((1.0 / s)).astype(np.float32)
    xt[:, D:] = xt[:, :D] * inv_n[:, None]

    b = float(beta[0])
    out_full = np.zeros((n_nodes, D), dtype=np.float32)
    shapes = pre['shapes']
    n_lo_rounds = pre['n_lo_rounds']

    for c in range(N_CORES):
        cc = pre['cores'][c]
        off_il = off_ih = off_bl = off_bh = 0
        outs = []
        for j, (kl, kh) in enumerate(shapes):
            own_hi = j >= n_lo_rounds
            nil, nih = P * T * kl, P * T * kh
            plane_l = cc['idx_lo'][:16, off_il:off_il + nil // 16]
            plane_h = cc['idx_hi'][:16, off_ih:off_ih + nih // 16]
            off_il += nil // 16; off_ih += nih // 16
            L_lo = plane_l.T.reshape(-1)
            L_hi = plane_h.T.reshape(-1)
            B_lo = cc['b_lo'][:, off_bl:off_bl + T * kl]; off_bl += T * kl
            B_hi = cc['b_hi'][:, off_bh:off_bh + T * kh]; off_bh += T * kh
            # gather: G[p, g, :] = slice[L[g*128+p]]
            G_lo = xt[:half][L_lo.reshape(T * kl, P).T.astype(np.int64)]
            G_hi = xt[half:][L_hi.reshape(T * kh, P).T.astype(np.int64)]
            G_lo = G_lo.reshape(P, T, kl, ELEM)
            G_hi = G_hi.reshape(P, T, kh, ELEM)
            G_own = G_hi if own_hi else G_lo
            xh_self = G_own[:, :, 0, D:]                      # [P, T, 32]
            x_self = G_own[:, :, 0, :D]
            dot_lo = (G_lo[:, :, :, D:] * xh_self[:, :, None, :]).sum(-1)
            dot_hi = (G_hi[:, :, :, D:] * xh_self[:, :, None, :]).sum(-1)
            a_lo = dot_lo + B_lo.reshape(P, T, kl)
            a_hi = dot_hi + B_hi.reshape(P, T, kh)
            e_lo = np.exp(b * a_lo - b)
            e_hi = np.exp(b * a_hi - b)
            den = e_lo.sum(-1) + e_hi.sum(-1)                 # [P, T]
            agg = (e_lo[..., None] * G_lo[:, :, :, :D]).sum(2) \
                + (e_hi[..., None] * G_hi[:, :, :, :D]).sum(2)
            o = x_self + np.maximum(agg / den[..., None], 0.0)
            outs.append(o.transpose(1, 0, 2).reshape(T * P, D))
        out_c = np.concatenate(outs, 0)
        m = cc['real']
        out_full[cc['nodes'][m]] = out_c[m]
    return out_full


# ---------------------------------------------------------------------------
# device program
# ---------------------------------------------------------------------------

_PROG_CACHE = {}


def build_program(shapes, n_lo_rounds, half, n_pad, w_stream):
    import concourse.bass as bass
    import concourse.bacc as bacc
    import concourse.tile as tile
    from concourse import mybir

    f32 = mybir.dt.float32
    i16 = mybir.dt.int16
    Alu = mybir.AluOpType
    Act = mybir.ActivationFunctionType
    n_rounds = len(shapes)
    n_out = n_rounds * T * P

    nc = bacc.Bacc()
    xt = nc.declare_dram_parameter('xt', [n_pad, ELEM], f32, isOutput=False)
    xc = nc.declare_dram_parameter('xc', [n_pad, D], f32, isOutput=False)
    stream = nc.declare_dram_parameter('stream', [P, w_stream], i16, isOutput=False)
    beta_in = nc.declare_dram_parameter('beta', [P, 1], f32, isOutput=False)
    out = nc.declare_dram_parameter('out', [n_out, D], f32, isOutput=True)

    NT = n_pad // P                            # t-cols per partition (e.g. 391)
    xt_c = xt[:].rearrange('(p t) e -> p (t e)', p=P)
    xc_c = xc[:].rearrange('(p t) d -> p (t d)', p=P)
    out_r = out[:].rearrange('(j p) d -> p j d', p=P)

    with tile.TileContext(nc) as tc:
        with tc.tile_pool(name='const', bufs=1) as cpool:
            beta_sb = cpool.tile([P, 1], f32)
            negb_sb = cpool.tile([P, 1], f32)
            nc.sync.dma_start(out=beta_sb[:], in_=beta_in[:])
            nc.vector.tensor_scalar_mul(out=negb_sb[:], in0=beta_sb[:], scalar1=-1.0)

            # ---- prologue: xh = x / sqrt(sum(x^2) + eps) into xt[:, 32:64]
            n_chunk = PRO_CHUNKS
            ct = -(-NT // n_chunk)
            with tc.tile_pool(name='pro', bufs=PRO_BUFS) as ppool:
                for c0 in range(0, NT, ct):
                    cw = min(ct, NT - c0)
                    xtile = ppool.tile([P, ct * D], f32, tag='xtile')
                    sq = ppool.tile([P, ct * D], f32, tag='sq')
                    s = ppool.tile([P, ct], f32, tag='s')
                    inv = ppool.tile([P, ct], f32, tag='inv')
                    xh = ppool.tile([P, ct * D], f32, tag='xh')
                    xv = xtile[:, :cw * D].rearrange('p (t d) -> p t d', d=D)
                    # packed x rows for partition p are contiguous in xc
                    nc.sync.dma_start(out=xv, in_=xc_c[:, c0 * D:(c0 + cw) * D])
                    nc.gpsimd.tensor_tensor(
                        out=sq[:, :cw * D].rearrange('p (t d) -> p t d', d=D),
                        in0=xv, in1=xv, op=Alu.mult)
                    nc.vector.tensor_reduce(
                        out=s[:, :cw],
                        in_=sq[:, :cw * D].rearrange('p (t d) -> p t d', d=D),
                        axis=mybir.AxisListType.X, op=Alu.add)
                    nc.vector.tensor_scalar_add(
                        out=s[:, :cw], in0=s[:, :cw], scalar1=1e-30)
                    nc.vector.reciprocal(out=inv[:, :cw], in_=s[:, :cw])
                    nc.scalar.activation(out=inv[:, :cw], in_=inv[:, :cw],
                                         func=Act.Sqrt)
                    xhv = xh[:, :cw * D].rearrange('p (t d) -> p t d', d=D)
                    nc.vector.tensor_tensor(
                        out=xhv,
                        in0=xv,
                        in1=inv[:, :cw].unsqueeze(2).broadcast_to([P, cw, D]),
                        op=Alu.mult)
                    # alternate the two HWDGE rings (SP / ACT) so the
                    # strided table writes don't serialize on one ring
                    weng = nc.scalar if (c0 // ct) % 2 else nc.sync
                    weng.dma_start(
                        out=xt_c[:].rearrange('p (t e) -> p t e', e=ELEM)
                            [:, c0:c0 + cw, D:ELEM],
                        in_=xhv)

            # ---- main loop over rounds
            off_st = 0
            with tc.tile_pool(name='stp', bufs=4) as stpool, \
                    tc.tile_pool(name='main', bufs=3) as mpool:
                for j, (kl, kh) in enumerate(shapes):
                    own_hi = j >= n_lo_rounds
                    nil, nih = P * T * kl, P * T * kh
                    wi_l, wi_h = nil // 16, nih // 16
                    w_rnd = wi_l + wi_h + 2 * T * (kl + kh)
                    st = stpool.tile([P, w_rnd], i16, tag='st')
                    nc.scalar.dma_start(out=st[:], in_=stream[:, off_st:off_st + w_rnd])
                    off_st += w_rnd
                    it_l = st[:, 0:wi_l]
                    it_h = st[:, wi_l:wi_l + wi_h]
                    o2 = wi_l + wi_h
                    bt_l = st[:, o2:o2 + 2 * T * kl].bitcast(f32)
                    bt_h = st[:, o2 + 2 * T * kl:o2 + 2 * T * (kl + kh)].bitcast(f32)

                    g_l = mpool.tile([P, T * kl * ELEM], f32, tag='g_l')
                    g_h = mpool.tile([P, T * kh * ELEM], f32, tag='g_h')
                    nc.gpsimd.dma_gather(
                        out_ap=g_l[:].rearrange('p (m e) -> p m e', e=ELEM),
                        in_ap=xt[0:half], idxs_ap=it_l,
                        num_idxs=nil, num_idxs_reg=nil, elem_size=ELEM,
                        single_packet=False)
                    nc.gpsimd.dma_gather(
                        out_ap=g_h[:].rearrange('p (m e) -> p m e', e=ELEM),
                        in_ap=xt[half:], idxs_ap=it_h,
                        num_idxs=nih, num_idxs_reg=nih, elem_size=ELEM,
                        single_packet=False)

                    gv_l = g_l[:].rearrange('p (t k e) -> p t k e', t=T, e=ELEM)
                    gv_h = g_h[:].rearrange('p (t k e) -> p t k e', t=T, e=ELEM)
                    gv_own = gv_h if own_hi else gv_l
                    k_own = kh if own_hi else kl

                    dm_dt = mybir.dt.bfloat16 if DM_BF16 else f32
                    dm_l = mpool.tile([P, T * kl * D], dm_dt, tag='dm_l')
                    dm_h = mpool.tile([P, T * kh * D], dm_dt, tag='dm_h')
                    a_l = mpool.tile([P, T * kl], f32, tag='a_l')
                    a_h = mpool.tile([P, T * kh], f32, tag='a_h')
                    e_l = mpool.tile([P, T * kl], f32, tag='e_l')
                    e_h = mpool.tile([P, T * kh], f32, tag='e_h')
                    den_l = mpool.tile([P, T], f32, tag='den_l')
                    den_h = mpool.tile([P, T], f32, tag='den_h')
                    rec = mpool.tile([P, T], f32, tag='rec')
                    p_l = mpool.tile([P, T * D * kl], dm_dt, tag='dm_l')
                    p_h = mpool.tile([P, T * D * kh], dm_dt, tag='dm_h')
                    ag_l = mpool.tile([P, T * D], f32, tag='ag_l')
                    ag_h = mpool.tile([P, T * D], f32, tag='ag_h')
                    osb = mpool.tile([P, T * D], f32, tag='osb')

                    for si, (gv, kk, dm, a, bt, ee, den) in enumerate((
                            (gv_l, kl, dm_l, a_l, bt_l, e_l, den_l),
                            (gv_h, kh, dm_h, a_h, bt_h, e_h, den_h))):
                        on_pool = f'dm{si}' in OFFLOAD_POOL and not (
                            si == 0 and (j % 5) < BALANCE_DM0_DVE)
                        mul_eng = nc.gpsimd if on_pool else nc.vector
                        self_xh = gv_own[:, :, 0:1, D:ELEM].broadcast_to(
                            [P, T, kk, D])
                        dmv = dm[:].rearrange('p (t k d) -> p t k d', t=T, d=D)
                        mul_eng.tensor_tensor(
                            out=dmv, in0=gv[:, :, :, D:ELEM], in1=self_xh,
                            op=Alu.mult)
                        nc.vector.tensor_reduce(
                            out=a[:].rearrange('p (t k) -> p t k', t=T),
                            in_=dmv, axis=mybir.AxisListType.X, op=Alu.add)
                        nc.vector.tensor_tensor(
                            out=a[:], in0=a[:], in1=bt, op=Alu.add)
                        for t in range(T):
                            nc.scalar.activation(
                                out=ee[:, t * kk:(t + 1) * kk],
                                in_=a[:, t * kk:(t + 1) * kk],
                                func=Act.Exp, bias=negb_sb[:], scale=beta_sb[:],
                                accum_out=den[:, t:t + 1])

                    nc.vector.tensor_tensor(out=rec[:], in0=den_l[:],
                                            in1=den_h[:], op=Alu.add)
                    nc.vector.reciprocal(out=rec[:], in_=rec[:])

                    for si, (gv, kk, ee, pp, ag) in enumerate((
                            (gv_l, kl, e_l, p_l, ag_l),
                            (gv_h, kh, e_h, p_h, ag_h))):
                        mul_eng = nc.gpsimd if f'p{si}' in OFFLOAD_POOL \
                            else nc.vector
                        ppv = pp[:].rearrange('p (t d k) -> p t d k', t=T, d=D)
                        mul_eng.tensor_tensor(
                            out=ppv,
                            in0=gv[:, :, :, 0:D].transpose([0, 1, 3, 2]),
                            in1=ee[:].rearrange('p (t k) -> p t k', t=T)
                                .unsqueeze(2).broadcast_to([P, T, D, kk]),
                            op=Alu.mult)
                        nc.vector.tensor_reduce(
                            out=ag[:].rearrange('p (t d) -> p t d', t=T),
                            in_=ppv, axis=mybir.AxisListType.X, op=Alu.add)

                    agv = ag_l[:].rearrange('p (t d) -> p t d', t=T)
                    nc.vector.tensor_tensor(
                        out=agv, in0=agv,
                        in1=ag_h[:].rearrange('p (t d) -> p t d', t=T), op=Alu.add)
                    nc.vector.tensor_tensor(
                        out=agv, in0=agv,
                        in1=rec[:].unsqueeze(2).broadcast_to([P, T, D]),
                        op=Alu.mult)
                    nc.vector.tensor_scalar_max(out=ag_l[:], in0=ag_l[:],
                                                scalar1=0.0)
                    nc.vector.tensor_tensor(
                        out=osb[:].rearrange('p (t d) -> p t d', t=T)
                            .unsqueeze(2),
                        in0=agv.unsqueeze(2),
                        in1=gv_own[:, :, 0:1, 0:D], op=Alu.add)
                    nc.sync.dma_start(
                        out=out_r[:, j * T:(j + 1) * T, :],
                        in_=osb[:].rearrange('p (t d) -> p t d', t=T))
    nc.finalize()
    return nc


def build_stream(pre, c):
    """Concatenate one core's per-round [idx_lo | idx_hi | b_lo | b_hi]
    planes into a single int16 stream (bias bitcast to 2x int16)."""
    cc = pre['cores'][c]
    parts = []
    off_il = off_ih = off_bl = off_bh = 0
    for j, (kl, kh) in enumerate(pre['shapes']):
        wi_l, wi_h = P * T * kl // 16, P * T * kh // 16
        parts.append(cc['idx_lo'][:, off_il:off_il + wi_l])
        parts.append(cc['idx_hi'][:, off_ih:off_ih + wi_h])
        parts.append(cc['b_lo'][:, off_bl:off_bl + T * kl].view(np.int16))
        parts.append(cc['b_hi'][:, off_bh:off_bh + T * kh].view(np.int16))
        off_il += wi_l; off_ih += wi_h
        off_bl += T * kl; off_bh += T * kh
    return np.ascontiguousarray(np.concatenate(parts, axis=1))


def kernel(x, edge_index, beta):
    from concourse.bass_utils import run_bass_kernel_spmd

    x = np.asarray(x, dtype=np.float32)
    edge_index = np.asarray(edge_index)
    beta = np.asarray(beta, dtype=np.float32)
    n_nodes = x.shape[0]

    pre = preprocess(edge_index, n_nodes)
    half, n_pad = pre['half'], pre['n_pad']
    shapes = pre['shapes']
    streams = [build_stream(pre, c) for c in range(N_CORES)]
    w_stream = streams[0].shape[1]

    key = (tuple(shapes), pre['n_lo_rounds'], half, n_pad, w_stream)
    nc = _PROG_CACHE.get(key)
    if nc is None:
        nc = build_program(shapes, pre['n_lo_rounds'], half, n_pad, w_stream)
        _PROG_CACHE[key] = nc

    xt = np.zeros((n_pad, ELEM), dtype=np.float32)
    xt[:n_nodes, :D] = x
    xc = np.zeros((n_pad, D), dtype=np.float32)
    xc[:n_nodes] = x
    beta_bc = np.tile(beta.reshape(1, 1), (P, 1)).astype(np.float32)

    in_maps = []
    for c in range(N_CORES):
        in_maps.append({'xt': xt, 'xc': xc, 'stream': streams[c],
                        'beta': beta_bc})
    res = run_bass_kernel_spmd(nc, in_maps, list(range(N_CORES)))

    out_full = np.zeros((n_nodes, D), dtype=np.float32)
    for c in range(N_CORES):
        cc = pre['cores'][c]
        o = res.results[c]['out']
        # device row (j*T + t)*128 + p holds node cc['nodes'][(j*T+t)*128 + p]?
        # nodes are stored per supertile in [t*128 + p] order already.
        m = cc['real']
        out_full[cc['nodes'][m]] = o[m]
    return out_full


if __name__ == '__main__':
    # quick self-check against a jax-free reference on small data
    rng = np.random.default_rng(1)
    n, e = 3000, 60000
    x = rng.standard_normal((n, D)).astype(np.float32)
    ei = rng.integers(0, n, size=(2, e)).astype(np.int64)
    beta = np.ones((1,), np.float32)

    def ref(x, edge_index, beta):
        N = x.shape[0]
        loops = np.arange(N, dtype=np.int64)
        s = np.concatenate([edge_index[0], loops])
        d = np.concatenate([edge_index[1], loops])
        nrm = np.sqrt((x * x).sum(-1, keepdims=True))
        xn = x / np.maximum(nrm, 1e-12)
        alpha = beta[0] * (xn[d] * xn[s]).sum(-1)
        amax = np.full(N, -np.inf)
        np.maximum.at(amax, d, alpha)
        ex = np.exp(alpha - amax[d])
        den = np.zeros(N)
        np.add.at(den, d, ex)
        w = ex / np.maximum(den[d], 1e-12)
        out = np.zeros_like(x)
        np.add.at(out, d, (w[:, None] * x[s]).astype(np.float32))
        return x + np.maximum(out, 0.0)

    want = ref(x, ei, beta)
    pre = preprocess(ei, n)
    got = emulate(x, beta, pre)
    err = np.abs(got - want).max() / np.abs(want).max()
    ks = np.array(pre['shapes'])
    tot_slots = (ks.sum(1) * P * T).sum() * N_CORES
    print('emulate rel err:', err)
    print('rounds:', len(pre['shapes']), 'slot amplification:',
          tot_slots / (e + n))
